# revision 38
# baseline (speedup 1.0000x reference)
"""Causal self-attention kernel for 8 Trainium2 NeuronCores.

Problem: B=4, T=2048, C=1024, H=16 heads (D=64).
Sharding: data-parallel over batch (4) x tensor-parallel over heads (2 groups
of 8 heads). Core c handles batch c//2, head-group c%2. Each core computes
qkv for its 8 heads, full causal attention on TxT scores, and its partial
projection output; the host sums the two head-group partials per batch.

Schedule (v2): t-chunk-outer pipeline. For each 512-wide row chunk ic the
core runs attention for all 4 head pairs; the qkv matmuls for chunk ic+1 and
the projection matmuls for chunk ic-1 are interleaved into the attention
stream at fixed slots so the PE never idles (keeps the DVFS p-state at max).
Scores are computed transposed (S.T = [j, i]); the causal mask is applied
multiplicatively (bf16, 4x DVE mode) after the exp; softmax normalization is
deferred: the PV matmul's ones-column yields l, 1/l comes from
reciprocal_approx_fast straight off PSUM, a PE ones-outer-product broadcasts
it, and a fused scalar_tensor_tensor does copy+normalize into SBUF.
PSUM->SBUF staging copies run on the otherwise idle GpSimd (Pool) engine.
"""

import sys

if "/opt/trn_rl_repo" not in sys.path:
    sys.path.insert(0, "/opt/trn_rl_repo")

from collections import deque
from contextlib import ExitStack

import ml_dtypes
import numpy as np

import concourse.bass as bass
import concourse.mybir as mybir
import concourse.tile as tile
from concourse.bass_utils import run_bass_kernel_spmd

BF16 = mybir.dt.bfloat16
F32 = mybir.dt.float32
F32R = mybir.dt.float32r
NP_BF16 = ml_dtypes.bfloat16

P = 128
B, T, C = 4, 2048, 1024
H = 16
D = 64
HL = 8            # heads per core
NPAIR = HL // 2   # head pairs per core
NL = HL * D       # 512: local qkv width
CT = C // P       # 8 contraction tiles over C
DT = NL // P      # 4 contraction tiles over local head dims
NTO = C // P      # 8 output tiles for proj
TCH = T // 512    # 4 t-chunks
NJT = T // P      # 16 j tiles


def _split_excess_waits(nc, limit=1):
    """This walrus build supports a single sem-wait per instruction; move
    excess waits emitted by Tile onto preceding same-engine NoOps."""
    n = 0
    for bb in nc.main_func.blocks:
        out = []
        changed = False
        for inst in bb.instructions:
            si = inst.sync_info
            if si is not None and len(si.on_wait) > limit:
                waits = list(si.on_wait)
                excess, keep = waits[:-limit], waits[-limit:]
                for i in range(0, len(excess), limit):
                    out.append(
                        mybir.InstNoOp(
                            name=f"waitsplit_{n}",
                            ins=[],
                            outs=[],
                            engine=inst.engine,
                            sync_info=mybir.SyncInfo(
                                on_wait=excess[i : i + limit], on_update=[]
                            ),
                        )
                    )
                    n += 1
                si.on_wait = keep
                changed = True
            out.append(inst)
        if changed:
            bb.instructions = out
    return n


def build_nc(with_bias=False, split_waits=True):
    nc = bass.Bass()
    AF = mybir.ActivationFunctionType
    ALU = mybir.AluOpType

    xT = nc.dram_tensor("xT", [P, TCH, CT, 512], BF16, kind="ExternalInput")
    wq = nc.dram_tensor("wq", [P, CT, NL], BF16, kind="ExternalInput")
    wk = nc.dram_tensor("wk", [P, CT, NL], BF16, kind="ExternalInput")
    wv = nc.dram_tensor("wv", [P, CT, NL], BF16, kind="ExternalInput")
    wp = nc.dram_tensor("wp", [P, DT, C], BF16, kind="ExternalInput")
    msk = nc.dram_tensor("msk", [P, P], BF16, kind="ExternalInput")
    if with_bias:
        bq = nc.dram_tensor("bq", [P, NPAIR], F32, kind="ExternalInput")
        bk = nc.dram_tensor("bk", [P, NPAIR], F32, kind="ExternalInput")
        bv = nc.dram_tensor("bv", [P, NL], F32, kind="ExternalInput")
        bp = nc.dram_tensor("bp", [P, NTO], F32, kind="ExternalInput")
    outT = nc.dram_tensor("outT", [P, NTO, T], F32, kind="ExternalOutput")

    with tile.TileContext(nc) as tc, ExitStack() as ctx:
        persist = ctx.enter_context(tc.tile_pool(name="persist", bufs=1))
        # PSUM: "s" tiles [128, 2, 512] (2 banks) x3 bufs = 6 banks; yA/yB
        # accumulators 1 bank each x1 buf = 2 banks. Total 8 banks.
        spsum = ctx.enter_context(tc.tile_pool(name="spsum", bufs=3, space="PSUM"))
        ypsum = ctx.enter_context(tc.tile_pool(name="ypsum", bufs=1, space="PSUM"))
        work = ctx.enter_context(tc.tile_pool(name="work", bufs=3))
        owork = ctx.enter_context(tc.tile_pool(name="owork", bufs=2))

        # ---- persistent SBUF tensors ----
        qT = persist.tile([P, NPAIR, T], BF16)   # [2x64d, pair, t]
        kT = persist.tile([P, NPAIR, T], BF16)
        vA = persist.tile([P, NJT, HL, D + 1], BF16)  # [j, jt, head, d|ones]
        yU = persist.tile([P, DT, T], BF16)  # normalized y.T pair-packed
        ones64 = persist.tile([1, D], F32)   # lhsT for PE partition-broadcast

        xs = persist.tile([P, TCH, CT, 512], BF16)
        wqs = persist.tile([P, CT, NL], BF16)
        wks = persist.tile([P, CT, NL], BF16)
        wvs = persist.tile([P, CT, NL], BF16)
        wps = persist.tile([P, DT, C], BF16)
        msks = persist.tile([P, 1, P], BF16)
        if with_bias:
            bqs = persist.tile([P, NPAIR], F32)
            bks = persist.tile([P, NPAIR], F32)
            bvs = persist.tile([P, NL], F32)
            bps = persist.tile([P, NTO], F32)

        # small tensors first, then weights in first-use order, x in
        # parallel on the other queue. wv and x chunk 0 are split by c-tile
        # so the very first emit_v matmul can start after ~160KB of DMA.
        nc.scalar.dma_start(msks[:, 0, :], msk[:])
        if with_bias:
            nc.scalar.dma_start(bqs[:], bq[:])
            nc.scalar.dma_start(bks[:], bk[:])
            nc.scalar.dma_start(bvs[:], bv[:])
            nc.scalar.dma_start(bps[:], bp[:])
        for cs in (slice(0, 2), slice(2, 8)):
            nc.scalar.dma_start(wvs[:, cs, :], wv[:, cs, :])
        for cs in (slice(0, 4), slice(4, 8)):
            nc.scalar.dma_start(wqs[:, cs, :], wq[:, cs, :])
            nc.scalar.dma_start(wks[:, cs, :], wk[:, cs, :])
        nc.scalar.dma_start(wps[:], wp[:])
        for cs in (slice(0, 2), slice(2, 8)):
            nc.sync.dma_start(xs[:, 0, cs, :], xT[:, 0, cs, :])
        for tc_i in range(1, TCH):
            nc.sync.dma_start(xs[:, tc_i, :, :], xT[:, tc_i, :, :])

        nc.vector.memset(vA[:, :, :, D : D + 1], 1.0)
        nc.vector.memset(ones64[:], 1.0)

        def xsl(ct, t0, n):  # slice of xs covering [t0, t0+n) at c-tile ct
            tc_i, o = divmod(t0, 512)
            return xs[:, tc_i, ct, o : o + n]

        def emit_v(tt):
            ps = spsum.tile([P, 2, 512], F32, tag="s")
            for ct in range(CT):
                nc.tensor.matmul(
                    ps[:, 0, :],
                    lhsT=xsl(ct, tt * P, P),
                    rhs=wvs[:, ct, :],
                    start=(ct == 0),
                    stop=(ct == CT - 1),
                )
            if with_bias:
                nc.vector.tensor_tensor(
                    out=vA[:, tt, :, 0:D],
                    in0=ps[:, 0, :].rearrange("p (h d) -> p h d", h=HL),
                    in1=bvs.rearrange("p (h d) -> p h d", h=HL),
                    op=ALU.add,
                )
            else:
                nc.vector.tensor_copy(
                    vA[:, tt, :, 0:D],
                    ps[:, 0, :].rearrange("p (h d) -> p h d", h=HL),
                )

        def emit_qk(nt, tc_i):
            ps = spsum.tile([P, 2, 512], F32, tag="s")
            t_sl = slice(tc_i * 512, (tc_i + 1) * 512)
            for ct in range(CT):
                nc.tensor.matmul(
                    ps[:, 0, :],
                    lhsT=wqs[:, ct, nt * P : (nt + 1) * P],
                    rhs=xs[:, tc_i, ct, :],
                    start=(ct == 0),
                    stop=(ct == CT - 1),
                )
            for ct in range(CT):
                nc.tensor.matmul(
                    ps[:, 1, :],
                    lhsT=wks[:, ct, nt * P : (nt + 1) * P],
                    rhs=xs[:, tc_i, ct, :],
                    start=(ct == 0),
                    stop=(ct == CT - 1),
                )
            if with_bias:
                nc.vector.tensor_scalar(
                    out=qT[:, nt, t_sl], in0=ps[:, 0, :],
                    scalar1=bqs[:, nt : nt + 1], scalar2=None, op0=ALU.add,
                )
                nc.vector.tensor_scalar(
                    out=kT[:, nt, t_sl], in0=ps[:, 1, :],
                    scalar1=bks[:, nt : nt + 1], scalar2=None, op0=ALU.add,
                )
            else:
                nc.vector.tensor_copy(qT[:, nt, t_sl], ps[:, 0, :])
                nc.vector.tensor_copy(kT[:, nt, t_sl], ps[:, 1, :])

        def proj_group(tc_i, g, nts=2):
            # projection for output tiles nt = nts*g .. on t-chunk tc_i
            ps = spsum.tile([P, 2, 512], F32, tag="s")
            t_sl = slice(tc_i * 512, (tc_i + 1) * 512)
            for h in range(nts):
                nt = nts * g + h
                for dt in range(DT):
                    nc.tensor.matmul(
                        ps[:, h, :],
                        lhsT=wps[:, dt, nt * P : (nt + 1) * P],
                        rhs=yU[:, dt, t_sl],
                        start=(dt == 0),
                        stop=(dt == DT - 1),
                    )
            ot = owork.tile([P, 2, 512], F32, tag="out")
            if with_bias:
                for h in range(nts):
                    nt = nts * g + h
                    nc.vector.tensor_scalar(
                        out=ot[:, h, :], in0=ps[:, h, :],
                        scalar1=bps[:, nt : nt + 1], scalar2=None, op0=ALU.add,
                    )
            elif nts == 1 and g % 2 == 1:
                nc.scalar.copy(ot[:, 0:1, :], ps[:, 0:1, :])
            else:
                nc.vector.tensor_copy(ot[:, 0:nts, :], ps[:, 0:nts, :])
            # output DMAs ride the otherwise-idle gpsimd queue so they never
            # delay the latency-critical l-chain DMAs on sync.
            nc.gpsimd.dma_start(
                outT[:, nts * g : nts * g + nts, t_sl], ot[:, 0:nts, :]
            )

        # deferred work-queue: qkv for next chunk / proj for prev chunk get
        # emitted inside the attention stream to keep the PE busy while the
        # scalar engine works through the exps.
        work_q = deque()

        def drain(n):
            for _ in range(n):
                if work_q:
                    work_q.popleft()()

        # ---- prologue: v tiles + pair-0 qk for chunk 0 ----
        for tt in range(4):
            emit_v(tt)
        emit_qk(0, 0)
        for pr in range(1, NPAIR):
            work_q.append(lambda pr=pr: emit_qk(pr, 0))

        def make_norm(ic, lr2):
            # reciprocal over all 8 lane-spread l rows, restage to partition
            # 0, then per-pair: PE ones-broadcast of 1/l into a full 128-row
            # tile and one in-place normalize multiply.
            i_sl = slice(ic * 512, (ic + 1) * 512)

            def norm():
                li2 = work.tile([P, 32], F32, tag="li2")
                nc.vector.reciprocal(li2[:], lr2[:])
                l3 = work.tile([1, 8, 512], F32, tag="l3")
                nc.sync.dma_start(l3[0:1, :, :], li2[:])
                for pr in range(NPAIR):
                    # l3 rows per pair are (head B, head A) — see tail DMA
                    lbt = spsum.tile([P, 2, 512], F32, tag="s")
                    nc.tensor.matmul(
                        lbt[0:D, 0, :], lhsT=ones64[:].bitcast(F32R),
                        rhs=l3[0:1, 2 * pr + 1, :].bitcast(F32R),
                        start=True, stop=True,
                    )
                    nc.tensor.matmul(
                        lbt[0:D, 1, :], lhsT=ones64[:].bitcast(F32R),
                        rhs=l3[0:1, 2 * pr, :].bitcast(F32R),
                        start=True, stop=True,
                    )
                    nc.vector.tensor_tensor(
                        out=yU[0:D, pr, i_sl], in0=yU[0:D, pr, i_sl],
                        in1=lbt[0:D, 0, :], op=ALU.mult,
                    )
                    nc.vector.tensor_tensor(
                        out=yU[D:P, pr, i_sl], in0=yU[D:P, pr, i_sl],
                        in1=lbt[0:D, 1, :], op=ALU.mult,
                    )

            return norm

        JT_SLOTS = [{1, 3}, {3, 7}, {5, 9}, {11}]
        pending_norm = None
        for ic in range(TCH):
            njt = 4 * ic + 4
            i0 = ic * 512
            i_sl = slice(i0, i0 + 512)
            if ic + 1 < TCH:
                for tt in range(4 * (ic + 1), 4 * (ic + 1) + 4):
                    work_q.append(lambda tt=tt: emit_v(tt))
                for pr in range(NPAIR):
                    work_q.append(lambda pr=pr: emit_qk(pr, ic + 1))
            if ic > 1:
                for g in range(4):
                    work_q.append(lambda g=g: proj_group(ic - 1, g))
            if ic == TCH - 1:
                # chunk 0's projection is saved for last: it fills the PE
                # during B(3)'s exp-bound stretch and the final norm chain.
                for g in range(4):
                    work_q.append(lambda g=g: proj_group(0, g))
            # at ic==3 attention itself nearly saturates the scalar engine;
            # only inject at pair boundaries there.
            jt_slots = JT_SLOTS[ic]
            lr2 = work.tile([P, 32], F32, tag="lr2")  # lane-spread l rows

            tiles = [(pr, jt) for pr in range(NPAIR) for jt in range(njt)]
            sts = {}
            yAB = {}

            def emit_scores(pr, jt):
                st = spsum.tile([P, 2, 512], F32, tag="s")
                sts[(pr, jt)] = st
                ow = max(0, jt * P - i0)
                j_sl = slice(jt * P, (jt + 1) * P)
                q_sl = slice(i0 + ow, i0 + 512)
                nc.tensor.matmul(
                    st[:, 0, ow:512],
                    lhsT=kT[0:D, pr, j_sl],
                    rhs=qT[0:D, pr, q_sl],
                    start=True, stop=True,
                    tile_position=(0, 0),
                )
                nc.tensor.matmul(
                    st[:, 1, ow:512],
                    lhsT=kT[D:P, pr, j_sl],
                    rhs=qT[D:P, pr, q_sl],
                    start=True, stop=True,
                    tile_position=(64, 0),
                )

            emit_scores(*tiles[0])
            emit_scores(*tiles[1])
            for idx, (pr, jt) in enumerate(tiles):
                if jt == 0:
                    if pr > 0:
                        drain(1)
                    yA = ypsum.tile([D + 1, 512], F32, tag="yA")
                    yB = ypsum.tile([D + 1, 512], F32, tag="yB")
                    yAB[pr] = (yA, yB)
                yA, yB = yAB[pr]
                st = sts.pop((pr, jt))
                ow = max(0, jt * P - i0)
                pt = work.tile([P, 2, 512], BF16, tag="p")
                nc.scalar.activation(
                    pt[:, :, ow:512], st[:, :, ow:512], AF.Exp, scale=0.125
                )
                if jt >= 4 * ic:  # diagonal tile: zero above-diag probs
                    nc.gpsimd.tensor_tensor(
                        out=pt[:, :, ow : ow + P],
                        in0=pt[:, :, ow : ow + P],
                        in1=msks[:].to_broadcast([P, 2, P]),
                        op=ALU.mult,
                    )
                if idx + 2 < len(tiles):
                    emit_scores(*tiles[idx + 2])
                nc.tensor.matmul(
                    yA[:, ow:512],
                    lhsT=vA[:, jt, 2 * pr, :],
                    rhs=pt[:, 0, ow:512],
                    start=(jt == 0),
                    stop=(jt == njt - 1),
                )
                nc.tensor.matmul(
                    yB[:, ow:512],
                    lhsT=vA[:, jt, 2 * pr + 1, :],
                    rhs=pt[:, 1, ow:512],
                    start=(jt == 0),
                    stop=(jt == njt - 1),
                )
                if pr == 0 and jt == 3 and pending_norm is not None:
                    pending_norm()
                    pending_norm = None
                if jt in jt_slots and (ic < TCH - 1 or pr < 2):
                    drain(1)
                if jt == njt - 1:
                    # pair tail: stash unnormalized y; extract the l
                    # ones-rows (head B's shifts partition 64 -> 0; the
                    # last pair's go via the now-idle scalar engine) and
                    # DMA both straight into the lane-spread lr2 slab in
                    # (B, A) row order.
                    lst = work.tile([P, 512], F32, tag="ls")
                    ceng = nc.scalar if pr == NPAIR - 1 else nc.vector
                    if pr == NPAIR - 1:
                        ceng.copy(lst[D : D + 1, :], yA[D : D + 1, :])
                        ceng.copy(lst[0:1, :], yB[D : D + 1, :])
                    else:
                        ceng.tensor_copy(lst[D : D + 1, :], yA[D : D + 1, :])
                        ceng.tensor_copy(lst[0:1, :], yB[D : D + 1, :])
                    nc.vector.tensor_copy(yU[0:D, pr, i_sl], yA[0:D, :])
                    nc.vector.tensor_copy(yU[D:P, pr, i_sl], yB[0:D, :])
                    r = 32 * pr
                    nc.sync.dma_start(
                        lr2[r : r + 32, :], lst[0:128:64, :]
                    )

            if pending_norm is not None:  # safety: ic 0 consumed none
                pending_norm()
            pending_norm = make_norm(ic, lr2)
            if ic == TCH - 1:
                drain(3)  # cover the final l-chain latency with proj work
                pending_norm()
                pending_norm = None
            drain(len(work_q))

        for g in range(NTO):
            proj_group(TCH - 1, g, nts=1)

    if split_waits:
        _split_excess_waits(nc, 1)
    return nc


def shard_inputs(x, w_attn, b_attn, w_proj, b_proj, with_bias):
    """Build the 8 per-core input dicts (core = 2*batch + head_group)."""
    x = np.asarray(x, dtype=np.float32)
    w_attn = np.asarray(w_attn, dtype=np.float32)
    b_attn = np.asarray(b_attn, dtype=np.float32)
    w_proj = np.asarray(w_proj, dtype=np.float32)
    b_proj = np.asarray(b_proj, dtype=np.float32)

    # multiplicative causal mask for a diagonal 128x128 block of S.T
    # ([j, i]): 1 where j <= i, 0 above the diagonal.
    pp = np.arange(P)
    msk = (pp[:, None] <= pp[None, :]).astype(NP_BF16)

    def wtile(w2d, ncols):  # [C_rows, ncols] -> [P, rows//P, ncols] bf16
        r = w2d.shape[0]
        return np.ascontiguousarray(
            w2d.reshape(r // P, P, ncols).transpose(1, 0, 2)
        ).astype(NP_BF16)

    in_maps = []
    for core in range(8):
        b, hg = divmod(core, 2)
        q0 = hg * NL
        xt = np.ascontiguousarray(x[b].T)  # [C, T]
        m = {
            "xT": np.ascontiguousarray(
                xt.reshape(CT, P, TCH, 512).transpose(1, 2, 0, 3)
            ).astype(NP_BF16),
            "wq": wtile(w_attn[:, q0 : q0 + NL], NL),
            "wk": wtile(w_attn[:, C + q0 : C + q0 + NL], NL),
            "wv": wtile(w_attn[:, 2 * C + q0 : 2 * C + q0 + NL], NL),
            "wp": wtile(w_proj[q0 : q0 + NL, :], C),
            "msk": msk,
        }
        if with_bias:
            m["bq"] = np.ascontiguousarray(
                b_attn[q0 : q0 + NL].reshape(NPAIR, P).T
            ).astype(np.float32)
            m["bk"] = np.ascontiguousarray(
                b_attn[C + q0 : C + q0 + NL].reshape(NPAIR, P).T
            ).astype(np.float32)
            m["bv"] = np.broadcast_to(
                b_attn[2 * C + q0 : 2 * C + q0 + NL], (P, NL)
            ).astype(np.float32)
            m["bp"] = (
                np.ascontiguousarray(b_proj.reshape(NTO, P).T).astype(np.float32)
                if hg == 0
                else np.zeros((P, NTO), np.float32)
            )
        in_maps.append(m)
    return in_maps


def unshard_output(results):
    """Combine 8 per-core outT [P, NTO, T] partials into [B, T, C] fp32."""
    out = np.empty((B, T, C), dtype=np.float32)
    for b in range(B):
        acc = results[2 * b]["outT"] + results[2 * b + 1]["outT"]
        # [P, NTO, T] -> [C, T] -> [T, C]
        out[b] = acc.transpose(1, 0, 2).reshape(C, T).T
    return out


_NC_CACHE = {}


def kernel(x, w_attn, b_attn, w_proj, b_proj):
    with_bias = bool(
        np.any(np.asarray(b_attn)) or np.any(np.asarray(b_proj))
    )
    key = ("nc", with_bias)
    if key not in _NC_CACHE:
        _NC_CACHE[key] = build_nc(with_bias)
    nc = _NC_CACHE[key]
    in_maps = shard_inputs(x, w_attn, b_attn, w_proj, b_proj, with_bias)
    res = run_bass_kernel_spmd(nc, in_maps, core_ids=list(range(8)))
    return unshard_output(res.results)


# revision 39
# speedup vs baseline: 1.0535x; 1.0535x over previous
"""Causal self-attention kernel for 8 Trainium2 NeuronCores.

Problem: B=4, T=2048, C=1024, H=16 heads (D=64).
Sharding: data-parallel over batch (4) x tensor-parallel over heads (2 groups
of 8 heads). Core c handles batch c//2, head-group c%2. Each core computes
qkv for its 8 heads, full causal attention on TxT scores, and its partial
projection output; the host sums the two head-group partials per batch.

Schedule (v2): t-chunk-outer pipeline. For each 512-wide row chunk ic the
core runs attention for all 4 head pairs; the qkv matmuls for chunk ic+1 and
the projection matmuls for chunk ic-1 are interleaved into the attention
stream at fixed slots so the PE never idles (keeps the DVFS p-state at max).
Scores are computed transposed (S.T = [j, i]); the causal mask is applied
multiplicatively (bf16, 4x DVE mode) after the exp; softmax normalization is
deferred: the PV matmul's ones-column yields l, 1/l comes from
reciprocal_approx_fast straight off PSUM, a PE ones-outer-product broadcasts
it, and a fused scalar_tensor_tensor does copy+normalize into SBUF.
PSUM->SBUF staging copies run on the otherwise idle GpSimd (Pool) engine.
"""

import sys

if "/opt/trn_rl_repo" not in sys.path:
    sys.path.insert(0, "/opt/trn_rl_repo")

from collections import deque
from contextlib import ExitStack

import ml_dtypes
import numpy as np

import concourse.bass as bass
import concourse.mybir as mybir
import concourse.tile as tile
from concourse.bass_utils import run_bass_kernel_spmd

BF16 = mybir.dt.bfloat16
F32 = mybir.dt.float32
F32R = mybir.dt.float32r
NP_BF16 = ml_dtypes.bfloat16

P = 128
B, T, C = 4, 2048, 1024
H = 16
D = 64
HL = 8            # heads per core
NPAIR = HL // 2   # head pairs per core
NL = HL * D       # 512: local qkv width
CT = C // P       # 8 contraction tiles over C
DT = NL // P      # 4 contraction tiles over local head dims
NTO = C // P      # 8 output tiles for proj
TCH = T // 512    # 4 t-chunks
NJT = T // P      # 16 j tiles


def _split_excess_waits(nc, limit=1):
    """This walrus build supports a single sem-wait per instruction; move
    excess waits emitted by Tile onto preceding same-engine NoOps."""
    n = 0
    for bb in nc.main_func.blocks:
        out = []
        changed = False
        for inst in bb.instructions:
            si = inst.sync_info
            if si is not None and len(si.on_wait) > limit:
                waits = list(si.on_wait)
                excess, keep = waits[:-limit], waits[-limit:]
                for i in range(0, len(excess), limit):
                    out.append(
                        mybir.InstNoOp(
                            name=f"waitsplit_{n}",
                            ins=[],
                            outs=[],
                            engine=inst.engine,
                            sync_info=mybir.SyncInfo(
                                on_wait=excess[i : i + limit], on_update=[]
                            ),
                        )
                    )
                    n += 1
                si.on_wait = keep
                changed = True
            out.append(inst)
        if changed:
            bb.instructions = out
    return n


def build_nc(with_bias=False, split_waits=True):
    nc = bass.Bass()
    AF = mybir.ActivationFunctionType
    ALU = mybir.AluOpType

    xT = nc.dram_tensor("xT", [P, TCH, CT, 512], BF16, kind="ExternalInput")
    wq = nc.dram_tensor("wq", [P, CT, NL], BF16, kind="ExternalInput")
    wk = nc.dram_tensor("wk", [P, CT, NL], BF16, kind="ExternalInput")
    wv = nc.dram_tensor("wv", [P, CT, NL], BF16, kind="ExternalInput")
    wp = nc.dram_tensor("wp", [P, DT, C], BF16, kind="ExternalInput")
    msk = nc.dram_tensor("msk", [P, P], BF16, kind="ExternalInput")
    if with_bias:
        bq = nc.dram_tensor("bq", [P, NPAIR], F32, kind="ExternalInput")
        bk = nc.dram_tensor("bk", [P, NPAIR], F32, kind="ExternalInput")
        bv = nc.dram_tensor("bv", [P, NL], F32, kind="ExternalInput")
        bp = nc.dram_tensor("bp", [P, NTO], F32, kind="ExternalInput")
    outT = nc.dram_tensor("outT", [P, NTO, T], F32, kind="ExternalOutput")

    with tile.TileContext(nc) as tc, ExitStack() as ctx:
        persist = ctx.enter_context(tc.tile_pool(name="persist", bufs=1))
        # PSUM: "s" tiles [128, 2, 512] (2 banks) x3 bufs = 6 banks; yA/yB
        # accumulators 1 bank each x1 buf = 2 banks. Total 8 banks.
        spsum = ctx.enter_context(tc.tile_pool(name="spsum", bufs=3, space="PSUM"))
        ypsum = ctx.enter_context(tc.tile_pool(name="ypsum", bufs=1, space="PSUM"))
        work = ctx.enter_context(tc.tile_pool(name="work", bufs=3))
        owork = ctx.enter_context(tc.tile_pool(name="owork", bufs=3))

        # ---- persistent SBUF tensors ----
        qT = persist.tile([P, NPAIR, T], BF16)   # [2x64d, pair, t]
        kT = persist.tile([P, NPAIR, T], BF16)
        vA = persist.tile([P, NJT, HL, D + 1], BF16)  # [j, jt, head, d|ones]
        yU = persist.tile([P, DT, T], BF16)  # normalized y.T pair-packed
        ones64 = persist.tile([1, D], F32)   # lhsT for PE partition-broadcast

        xs = persist.tile([P, TCH, CT, 512], BF16)
        wqs = persist.tile([P, CT, NL], BF16)
        wks = persist.tile([P, CT, NL], BF16)
        wvs = persist.tile([P, CT, NL], BF16)
        wps = persist.tile([P, DT, C], BF16)
        msks = persist.tile([P, 1, P], BF16)
        if with_bias:
            bqs = persist.tile([P, NPAIR], F32)
            bks = persist.tile([P, NPAIR], F32)
            bvs = persist.tile([P, NL], F32)
            bps = persist.tile([P, NTO], F32)

        # small tensors first, then weights in first-use order, x in
        # parallel on the other queue. wv and x chunk 0 are split by c-tile
        # so the very first emit_v matmul can start after ~160KB of DMA.
        nc.scalar.dma_start(msks[:, 0, :], msk[:])
        if with_bias:
            nc.scalar.dma_start(bqs[:], bq[:])
            nc.scalar.dma_start(bks[:], bk[:])
            nc.scalar.dma_start(bvs[:], bv[:])
            nc.scalar.dma_start(bps[:], bp[:])
        for cs in (slice(0, 2), slice(2, 8)):
            nc.scalar.dma_start(wvs[:, cs, :], wv[:, cs, :])
        for cs in (slice(0, 4), slice(4, 8)):
            nc.scalar.dma_start(wqs[:, cs, :], wq[:, cs, :])
            nc.scalar.dma_start(wks[:, cs, :], wk[:, cs, :])
        nc.scalar.dma_start(wps[:], wp[:])
        for cs in (slice(0, 2), slice(2, 8)):
            nc.sync.dma_start(xs[:, 0, cs, :], xT[:, 0, cs, :])
        for tc_i in range(1, TCH):
            nc.sync.dma_start(xs[:, tc_i, :, :], xT[:, tc_i, :, :])

        nc.vector.memset(vA[:, :, :, D : D + 1], 1.0)
        nc.vector.memset(ones64[:], 1.0)

        def xsl(ct, t0, n):  # slice of xs covering [t0, t0+n) at c-tile ct
            tc_i, o = divmod(t0, 512)
            return xs[:, tc_i, ct, o : o + n]

        def emit_v(tt):
            ps = spsum.tile([P, 2, 512], F32, tag="s")
            for ct in range(CT):
                nc.tensor.matmul(
                    ps[:, 0, :],
                    lhsT=xsl(ct, tt * P, P),
                    rhs=wvs[:, ct, :],
                    start=(ct == 0),
                    stop=(ct == CT - 1),
                )
            if with_bias:
                nc.vector.tensor_tensor(
                    out=vA[:, tt, :, 0:D],
                    in0=ps[:, 0, :].rearrange("p (h d) -> p h d", h=HL),
                    in1=bvs.rearrange("p (h d) -> p h d", h=HL),
                    op=ALU.add,
                )
            else:
                nc.vector.tensor_copy(
                    vA[:, tt, :, 0:D],
                    ps[:, 0, :].rearrange("p (h d) -> p h d", h=HL),
                )

        def emit_qk(nt, tc_i):
            ps = spsum.tile([P, 2, 512], F32, tag="s")
            t_sl = slice(tc_i * 512, (tc_i + 1) * 512)
            for ct in range(CT):
                nc.tensor.matmul(
                    ps[:, 0, :],
                    lhsT=wqs[:, ct, nt * P : (nt + 1) * P],
                    rhs=xs[:, tc_i, ct, :],
                    start=(ct == 0),
                    stop=(ct == CT - 1),
                )
            for ct in range(CT):
                nc.tensor.matmul(
                    ps[:, 1, :],
                    lhsT=wks[:, ct, nt * P : (nt + 1) * P],
                    rhs=xs[:, tc_i, ct, :],
                    start=(ct == 0),
                    stop=(ct == CT - 1),
                )
            if with_bias:
                nc.vector.tensor_scalar(
                    out=qT[:, nt, t_sl], in0=ps[:, 0, :],
                    scalar1=bqs[:, nt : nt + 1], scalar2=None, op0=ALU.add,
                )
                nc.vector.tensor_scalar(
                    out=kT[:, nt, t_sl], in0=ps[:, 1, :],
                    scalar1=bks[:, nt : nt + 1], scalar2=None, op0=ALU.add,
                )
            else:
                nc.vector.tensor_copy(qT[:, nt, t_sl], ps[:, 0, :])
                nc.vector.tensor_copy(kT[:, nt, t_sl], ps[:, 1, :])

        def proj_group(tc_i, g, nts=2):
            # projection for output tiles nt = nts*g .. on t-chunk tc_i
            ps = spsum.tile([P, 2, 512], F32, tag="s")
            t_sl = slice(tc_i * 512, (tc_i + 1) * 512)
            for h in range(nts):
                nt = nts * g + h
                for dt in range(DT):
                    nc.tensor.matmul(
                        ps[:, h, :],
                        lhsT=wps[:, dt, nt * P : (nt + 1) * P],
                        rhs=yU[:, dt, t_sl],
                        start=(dt == 0),
                        stop=(dt == DT - 1),
                    )
            ot = owork.tile([P, 2, 512], F32, tag="out")
            if with_bias:
                for h in range(nts):
                    nt = nts * g + h
                    nc.vector.tensor_scalar(
                        out=ot[:, h, :], in0=ps[:, h, :],
                        scalar1=bps[:, nt : nt + 1], scalar2=None, op0=ALU.add,
                    )
            elif nts == 1 and g % 2 == 1:
                nc.scalar.copy(ot[:, 0:1, :], ps[:, 0:1, :])
            else:
                nc.vector.tensor_copy(ot[:, 0:nts, :], ps[:, 0:nts, :])
            # output DMAs ride the otherwise-idle gpsimd queue so they
            # don't delay the latency-critical l-chain DMAs on sync; the
            # final thin groups alternate with sync (input queue is empty).
            eng = nc.sync if (nts == 1 and g % 2 == 1) else nc.gpsimd
            eng.dma_start(
                outT[:, nts * g : nts * g + nts, t_sl], ot[:, 0:nts, :]
            )

        # deferred work-queue: qkv for next chunk / proj for prev chunk get
        # emitted inside the attention stream to keep the PE busy while the
        # scalar engine works through the exps.
        work_q = deque()

        def drain(n):
            for _ in range(n):
                if work_q:
                    work_q.popleft()()

        # ---- prologue: v tiles + pair-0 qk for chunk 0 ----
        for tt in range(4):
            emit_v(tt)
        emit_qk(0, 0)
        for pr in range(1, NPAIR):
            work_q.append(lambda pr=pr: emit_qk(pr, 0))

        def make_norm(ic, lr2):
            # reciprocal over all 8 lane-spread l rows, restage to partition
            # 0, then per-pair: PE ones-broadcast of 1/l into a full 128-row
            # tile and one in-place normalize multiply.
            i_sl = slice(ic * 512, (ic + 1) * 512)

            def norm():
                li2 = work.tile([32, P], F32, tag="li2")
                nc.vector.reciprocal(li2[:], lr2[:])
                l3 = work.tile([1, 8, 512], F32, tag="l3")
                nc.sync.dma_start(l3[0:1, :, :], li2[:])
                for pr in range(NPAIR):
                    # l3 rows per pair are (head B, head A) — see tail DMA
                    lbt = spsum.tile([P, 2, 512], F32, tag="s")
                    nc.tensor.matmul(
                        lbt[0:D, 0, :], lhsT=ones64[:].bitcast(F32R),
                        rhs=l3[0:1, 2 * pr + 1, :].bitcast(F32R),
                        start=True, stop=True,
                    )
                    nc.tensor.matmul(
                        lbt[0:D, 1, :], lhsT=ones64[:].bitcast(F32R),
                        rhs=l3[0:1, 2 * pr, :].bitcast(F32R),
                        start=True, stop=True,
                    )
                    nc.vector.tensor_tensor(
                        out=yU[0:D, pr, i_sl], in0=yU[0:D, pr, i_sl],
                        in1=lbt[0:D, 0, :], op=ALU.mult,
                    )
                    nc.vector.tensor_tensor(
                        out=yU[D:P, pr, i_sl], in0=yU[D:P, pr, i_sl],
                        in1=lbt[0:D, 1, :], op=ALU.mult,
                    )

            return norm

        JT_SLOTS = [{1, 3}, {3, 7}, {5, 9}, {11}]
        pending_norm = None
        for ic in range(TCH):
            njt = 4 * ic + 4
            i0 = ic * 512
            i_sl = slice(i0, i0 + 512)
            if ic + 1 < TCH:
                for tt in range(4 * (ic + 1), 4 * (ic + 1) + 4):
                    work_q.append(lambda tt=tt: emit_v(tt))
                for pr in range(NPAIR):
                    work_q.append(lambda pr=pr: emit_qk(pr, ic + 1))
            if ic > 1:
                for g in range(4):
                    work_q.append(lambda g=g: proj_group(ic - 1, g))
            if ic == TCH - 1:
                # chunk 0's projection is saved for last: it fills the PE
                # during B(3)'s exp-bound stretch and the final norm chain.
                for g in range(4):
                    work_q.append(lambda g=g: proj_group(0, g))
            # at ic==3 attention itself nearly saturates the scalar engine;
            # only inject at pair boundaries there.
            jt_slots = JT_SLOTS[ic]
            lr2 = work.tile([32, P], F32, tag="lr2")  # lane-spread l rows

            tiles = [(pr, jt) for pr in range(NPAIR) for jt in range(njt)]
            sts = {}
            yAB = {}

            def emit_scores(pr, jt):
                st = spsum.tile([P, 2, 512], F32, tag="s")
                sts[(pr, jt)] = st
                ow = max(0, jt * P - i0)
                j_sl = slice(jt * P, (jt + 1) * P)
                q_sl = slice(i0 + ow, i0 + 512)
                nc.tensor.matmul(
                    st[:, 0, ow:512],
                    lhsT=kT[0:D, pr, j_sl],
                    rhs=qT[0:D, pr, q_sl],
                    start=True, stop=True,
                    tile_position=(0, 0),
                )
                nc.tensor.matmul(
                    st[:, 1, ow:512],
                    lhsT=kT[D:P, pr, j_sl],
                    rhs=qT[D:P, pr, q_sl],
                    start=True, stop=True,
                    tile_position=(64, 0),
                )

            emit_scores(*tiles[0])
            emit_scores(*tiles[1])
            for idx, (pr, jt) in enumerate(tiles):
                if jt == 0:
                    if pr > 0:
                        drain(1)
                    yA = ypsum.tile([D + 1, 512], F32, tag="yA")
                    yB = ypsum.tile([D + 1, 512], F32, tag="yB")
                    yAB[pr] = (yA, yB)
                yA, yB = yAB[pr]
                st = sts.pop((pr, jt))
                ow = max(0, jt * P - i0)
                pt = work.tile([P, 2, 512], BF16, tag="p")
                nc.scalar.activation(
                    pt[:, :, ow:512], st[:, :, ow:512], AF.Exp, scale=0.125
                )
                if jt >= 4 * ic:  # diagonal tile: zero above-diag probs
                    nc.vector.tensor_tensor(
                        out=pt[:, :, ow : ow + P],
                        in0=pt[:, :, ow : ow + P],
                        in1=msks[:].to_broadcast([P, 2, P]),
                        op=ALU.mult,
                    )
                if idx + 2 < len(tiles):
                    emit_scores(*tiles[idx + 2])
                nc.tensor.matmul(
                    yA[:, ow:512],
                    lhsT=vA[:, jt, 2 * pr, :],
                    rhs=pt[:, 0, ow:512],
                    start=(jt == 0),
                    stop=(jt == njt - 1),
                )
                nc.tensor.matmul(
                    yB[:, ow:512],
                    lhsT=vA[:, jt, 2 * pr + 1, :],
                    rhs=pt[:, 1, ow:512],
                    start=(jt == 0),
                    stop=(jt == njt - 1),
                )
                if pr == 0 and jt == 3 and pending_norm is not None:
                    pending_norm()
                    pending_norm = None
                if jt in jt_slots and (ic < TCH - 1 or pr < 2):
                    drain(1)
                if jt == njt - 1:
                    # pair tail: stash unnormalized y; extract the l
                    # ones-rows (head B's shifts partition 64 -> 0; the
                    # last pair's go via the now-idle scalar engine) and
                    # DMA both straight into the lane-spread lr2 slab in
                    # (B, A) row order.
                    lst = work.tile([P, 512], F32, tag="ls")
                    ceng = nc.scalar if pr == NPAIR - 1 else nc.vector
                    if pr == NPAIR - 1:
                        ceng.copy(lst[D : D + 1, :], yA[D : D + 1, :])
                        ceng.copy(lst[0:1, :], yB[D : D + 1, :])
                    else:
                        ceng.tensor_copy(lst[D : D + 1, :], yA[D : D + 1, :])
                        ceng.tensor_copy(lst[0:1, :], yB[D : D + 1, :])
                    nc.vector.tensor_copy(yU[0:D, pr, i_sl], yA[0:D, :])
                    nc.vector.tensor_copy(yU[D:P, pr, i_sl], yB[0:D, :])
                    r = 8 * pr
                    nc.sync.dma_start(
                        lr2[r : r + 8, :], lst[0:128:64, :]
                    )

            if pending_norm is not None:  # safety: ic 0 consumed none
                pending_norm()
            pending_norm = make_norm(ic, lr2)
            if ic == TCH - 1:
                drain(3)  # cover the final l-chain latency with proj work
                pending_norm()
                pending_norm = None
            drain(len(work_q))

        for g in range(NTO):
            proj_group(TCH - 1, g, nts=1)

    if split_waits:
        _split_excess_waits(nc, 1)
    return nc


def shard_inputs(x, w_attn, b_attn, w_proj, b_proj, with_bias):
    """Build the 8 per-core input dicts (core = 2*batch + head_group)."""
    x = np.asarray(x, dtype=np.float32)
    w_attn = np.asarray(w_attn, dtype=np.float32)
    b_attn = np.asarray(b_attn, dtype=np.float32)
    w_proj = np.asarray(w_proj, dtype=np.float32)
    b_proj = np.asarray(b_proj, dtype=np.float32)

    # multiplicative causal mask for a diagonal 128x128 block of S.T
    # ([j, i]): 1 where j <= i, 0 above the diagonal.
    pp = np.arange(P)
    msk = (pp[:, None] <= pp[None, :]).astype(NP_BF16)

    def wtile(w2d, ncols):  # [C_rows, ncols] -> [P, rows//P, ncols] bf16
        r = w2d.shape[0]
        return np.ascontiguousarray(
            w2d.reshape(r // P, P, ncols).transpose(1, 0, 2)
        ).astype(NP_BF16)

    in_maps = []
    for core in range(8):
        b, hg = divmod(core, 2)
        q0 = hg * NL
        xt = np.ascontiguousarray(x[b].T)  # [C, T]
        m = {
            "xT": np.ascontiguousarray(
                xt.reshape(CT, P, TCH, 512).transpose(1, 2, 0, 3)
            ).astype(NP_BF16),
            "wq": wtile(w_attn[:, q0 : q0 + NL], NL),
            "wk": wtile(w_attn[:, C + q0 : C + q0 + NL], NL),
            "wv": wtile(w_attn[:, 2 * C + q0 : 2 * C + q0 + NL], NL),
            "wp": wtile(w_proj[q0 : q0 + NL, :], C),
            "msk": msk,
        }
        if with_bias:
            m["bq"] = np.ascontiguousarray(
                b_attn[q0 : q0 + NL].reshape(NPAIR, P).T
            ).astype(np.float32)
            m["bk"] = np.ascontiguousarray(
                b_attn[C + q0 : C + q0 + NL].reshape(NPAIR, P).T
            ).astype(np.float32)
            m["bv"] = np.broadcast_to(
                b_attn[2 * C + q0 : 2 * C + q0 + NL], (P, NL)
            ).astype(np.float32)
            m["bp"] = (
                np.ascontiguousarray(b_proj.reshape(NTO, P).T).astype(np.float32)
                if hg == 0
                else np.zeros((P, NTO), np.float32)
            )
        in_maps.append(m)
    return in_maps


def unshard_output(results):
    """Combine 8 per-core outT [P, NTO, T] partials into [B, T, C] fp32."""
    out = np.empty((B, T, C), dtype=np.float32)
    for b in range(B):
        acc = results[2 * b]["outT"] + results[2 * b + 1]["outT"]
        # [P, NTO, T] -> [C, T] -> [T, C]
        out[b] = acc.transpose(1, 0, 2).reshape(C, T).T
    return out


_NC_CACHE = {}


def kernel(x, w_attn, b_attn, w_proj, b_proj):
    with_bias = bool(
        np.any(np.asarray(b_attn)) or np.any(np.asarray(b_proj))
    )
    key = ("nc", with_bias)
    if key not in _NC_CACHE:
        _NC_CACHE[key] = build_nc(with_bias)
    nc = _NC_CACHE[key]
    in_maps = shard_inputs(x, w_attn, b_attn, w_proj, b_proj, with_bias)
    res = run_bass_kernel_spmd(nc, in_maps, core_ids=list(range(8)))
    return unshard_output(res.results)
